# revision 1
# baseline (speedup 1.0000x reference)
"""DecoderLSTM Trainium2 kernel.

Problem: N=32768 batch rows, H=512 hidden, O=2 outputs, T=12 steps.
    h,c,x = (context, 0, start_pos)
    per step: gates = x @ W_ih.T + h @ W_hh.T + (b_ih + b_hh)
              i,f,g,o = split(gates); c = sig(f)*c + sig(i)*tanh(g)
              h = sig(o)*tanh(c); pred = h @ W_fc.T + b_fc; x = pred
    output: preds stacked [N, T, O]

Key algebraic fold: x_t = W_fc @ h_t + b_fc for t>=1, so
    gates_t = (W_hh + W_ih W_fc) @ h_{t-1} + (b + W_ih b_fc)   (t >= 1)
    gates_0 = W_hh @ context + W_ih @ start_pos + b
which removes the fed-back x input entirely (rank-2 weight correction,
done once on the host).

Sharding: pure data parallel, 4096 batch rows per core, weights replicated.

On-chip layout is "transposed": H lives on partitions (4 chunks of 128),
batch on the free dim, so the recurrence needs no transposes at all:
    gates.T[4H, B] = W_eff @ h.T  via matmul(lhsT=W_eff.T chunk, rhs=h.T chunk)
Per-gate bias is applied for free via the ACT engine's per-partition bias
on the sigmoid/tanh evacuation of PSUM.

Each core processes its 4096 batch cols as 4 chunks of 1024; two chunks
("chains") are interleaved so one chain's matmuls hide the other chain's
activation/elementwise tail between steps.
"""

import os

import numpy as np

import concourse.mybir as mybir
from concourse import bacc
import concourse.tile as tile
from concourse.bass_utils import run_bass_kernel_spmd

P = 128
H = 512
HC = H // P          # 4 H-chunks
G4 = 4 * H           # 2048 gate rows
O = 2
T = 12
N_FULL = 32768
N_CORES = 8
NB = N_FULL // N_CORES   # 4096 batch rows per core
BC = 1024                # batch cols per chain
NCH = NB // BC           # 4 chains per core
F32 = mybir.dt.float32
# float16 runs the PE at full rate (1 col/cycle) with overlapped FWL weight
# loads, and its 10-bit mantissa keeps the recurrence error ~4e-4 of scale.
# float32r (same speed on paper) measured ~20% slower on HW: its self-loading
# weight path serializes against the matmul stream. float32 is exact but 4x.
_MM_DT_NAME = os.environ.get("KERNEL_MM_DT", "float16")
MM_DT = getattr(mybir.dt, _MM_DT_NAME)

# Moving-operand free dim per matmul: one 512-entry PSUM bank.
MMN = 512
NHALF = BC // MMN

# Buffer depths (SBUF is ample at fp16): h ping-pong depth also controls how
# early the next pair's h0 DMA can start; act tiles decouple ACT from DVE.
H_BUFS = int(os.environ.get("KERNEL_HBUFS", "2"))
A_BUFS = int(os.environ.get("KERNEL_ABUFS", "2"))

SIG = mybir.ActivationFunctionType.Sigmoid
TANH = mybir.ActivationFunctionType.Tanh


def build_bass():
    # Bacc (not plain Bass): its compile() runs move_matmul_waits_to_ldweights
    # + generate_event_semaphores, which split semaphore waits to the 1-per-
    # instruction hardware limit. Without it, walrus rejects any 2-dep
    # instruction with 'Too many sync wait commands'.
    nc = bacc.Bacc()

    # Tensors feeding the PE are declared in MM_DT end-to-end (for fp32r
    # the BIR verifier additionally requires matmul operands to be *produced*
    # as fp32r, i.e. rounded at the producer).
    h0_d = nc.dram_tensor("h0", [P, HC, NB], MM_DT, kind="ExternalInput")
    sp_d = nc.dram_tensor("sp", [P, NB], MM_DT, kind="ExternalInput")
    wg_d = nc.dram_tensor("wg", [P, HC, G4], MM_DT, kind="ExternalInput")
    wx_d = nc.dram_tensor("wx", [P, G4], MM_DT, kind="ExternalInput")
    wf_d = nc.dram_tensor("wf", [P, HC, O], MM_DT, kind="ExternalInput")
    b0_d = nc.dram_tensor("b0", [P, G4 // P], F32, kind="ExternalInput")
    b1_d = nc.dram_tensor("b1", [P, G4 // P], F32, kind="ExternalInput")
    bfc_d = nc.dram_tensor("bfc", [O, 1], F32, kind="ExternalInput")
    out_d = nc.dram_tensor("preds", [2 * T, NB], F32, kind="ExternalOutput")

    with tile.TileContext(nc) as tc:
        with (
            tc.tile_pool(name="singles", bufs=1) as singles,
            tc.tile_pool(name="state", bufs=1) as state,
            tc.tile_pool(name="acts", bufs=2) as acts,
            tc.tile_pool(name="gpsum", bufs=3, space="PSUM") as gpsum,
            tc.tile_pool(name="ppsum", bufs=1, space="PSUM") as ppsum,
        ):
            wg_sb = singles.tile([P, HC, G4], MM_DT)
            # split by K-chunk so the first matmuls start after ~1/4 of the
            # weights have landed (the only PE idle in the modeled trace is
            # ~16us at startup waiting for these loads)
            for _kj in range(HC):
                nc.sync.dma_start(wg_sb[:, _kj, :], wg_d[:, _kj, :])
            wx_sb = singles.tile([P, G4], MM_DT)
            nc.sync.dma_start(wx_sb[:], wx_d[:])
            wf_sb = singles.tile([P, HC, O], MM_DT)
            nc.sync.dma_start(wf_sb[:], wf_d[:])
            sp_sb = singles.tile([P, NB], MM_DT)
            nc.sync.dma_start(sp_sb[:], sp_d[:])
            b0_sb = singles.tile([P, G4 // P], F32)
            nc.sync.dma_start(b0_sb[:], b0_d[:])
            b1_sb = singles.tile([P, G4 // P], F32)
            nc.sync.dma_start(b1_sb[:], b1_d[:])
            bfc_sb = singles.tile([O, 1], F32)
            nc.sync.dma_start(bfc_sb[:], bfc_d[:])

            st = {}

            def emit_step(s, t):
                """Gates + elementwise for chain s, step t. Updates st[s]."""
                h_prev, c = st[s]
                h_next = state.tile([P, HC, BC], MM_DT, tag=f"h{s % 2}",
                                    bufs=H_BUFS)
                b_sb = b0_sb if t == 0 else b1_sb
                for j in range(HC):
                    gsb = []
                    for g in range(4):
                        mb = 4 * g + j
                        msl = slice(mb * P, (mb + 1) * P)
                        gp = gpsum.tile([P, BC], F32, tag="g")
                        for half in range(NHALF):
                            cs = slice(half * MMN, (half + 1) * MMN)
                            for kj in range(HC):
                                nc.tensor.matmul(
                                    gp[:, cs],
                                    wg_sb[:, kj, msl],
                                    h_prev[:, kj, cs],
                                    start=(kj == 0),
                                    stop=(kj == HC - 1 and t > 0),
                                )
                            if t == 0:
                                scs = slice(s * BC + half * MMN,
                                            s * BC + (half + 1) * MMN)
                                nc.tensor.matmul(
                                    gp[:, cs],
                                    wx_sb[:, msl],
                                    sp_sb[:, scs],
                                    start=False,
                                    stop=True,
                                )
                        a = acts.tile([P, BC], F32, tag=f"a{g}", bufs=A_BUFS)
                        nc.scalar.activation(
                            a[:], gp[:],
                            TANH if g == 2 else SIG,
                            bias=b_sb[:, mb:mb + 1],
                        )
                        gsb.append(a)
                    ai, af, ag, ao = gsb
                    # c_j = f * c_j + i * g ; h_j = o * tanh(c_j)
                    nc.vector.tensor_mul(ai[:], ai[:], ag[:])          # i*g -> ai
                    nc.vector.tensor_mul(c[:, j, :], af[:], c[:, j, :])
                    nc.vector.tensor_add(c[:, j, :], c[:, j, :], ai[:])
                    nc.scalar.activation(ag[:], c[:, j, :], TANH)      # tanh(c) -> ag
                    nc.vector.tensor_mul(h_next[:, j, :], ao[:], ag[:])
                st[s] = (h_next, c)

            def emit_pred(s, t):
                """pred_t = W_fc @ h.T + b_fc -> DRAM rows [2t, 2t+2)."""
                h_cur = st[s][0]
                pp = ppsum.tile([O, NHALF, MMN], F32, tag="pred")
                for half in range(NHALF):
                    cs = slice(half * MMN, (half + 1) * MMN)
                    for kj in range(HC):
                        nc.tensor.matmul(
                            pp[:, half, :],
                            wf_sb[:, kj, :],
                            h_cur[:, kj, cs],
                            start=(kj == 0),
                            stop=(kj == HC - 1),
                        )
                psb = acts.tile([O, NHALF, MMN], F32, tag="pred_sb", bufs=1)
                nc.vector.tensor_scalar_add(psb[:], pp[:], bfc_sb[:, 0:1])
                dst = out_d[2 * t:2 * t + 2, s * BC:(s + 1) * BC]
                nc.sync.dma_start(
                    dst.rearrange("p (h x) -> p h x", h=NHALF), psb[:])

            for pair in range(NCH // 2):
                chains = [2 * pair, 2 * pair + 1]
                for s in chains:
                    h = state.tile([P, HC, BC], MM_DT, tag=f"h{s % 2}",
                                   bufs=H_BUFS)
                    for _kj in range(HC):
                        nc.sync.dma_start(
                            h[:, _kj, :],
                            h0_d[:, _kj, s * BC:(s + 1) * BC])
                    c = state.tile([P, HC, BC], F32, tag=f"c{s % 2}", bufs=1)
                    nc.vector.memset(c[:], 0.0)
                    st[s] = (h, c)
                a, b = chains
                for t in range(T):
                    emit_step(a, t)
                    if t > 0:
                        emit_pred(b, t - 1)
                    emit_step(b, t)
                    emit_pred(a, t)
                emit_pred(b, T - 1)

    nc.compile()
    return nc


_NC_CACHE = {}


def _get_nc():
    key = _MM_DT_NAME
    if key not in _NC_CACHE:
        _NC_CACHE[key] = build_bass()
    return _NC_CACHE[key]


MM_NP = mybir.dt.np(MM_DT)


def prepare_in_maps(inputs):
    ctx = np.ascontiguousarray(np.asarray(inputs["context"], dtype=np.float32))
    sp = np.ascontiguousarray(np.asarray(inputs["start_pos"], dtype=np.float32))
    W_ih = np.asarray(inputs["W_ih"], dtype=np.float32)
    W_hh = np.asarray(inputs["W_hh"], dtype=np.float32)
    b_ih = np.asarray(inputs["b_ih"], dtype=np.float32)
    b_hh = np.asarray(inputs["b_hh"], dtype=np.float32)
    W_fc = np.asarray(inputs["W_fc"], dtype=np.float32)
    b_fc = np.asarray(inputs["b_fc"], dtype=np.float32)

    # Fold the fed-back fc layer into the recurrence (exact algebra; done in
    # fp64 to keep the fold itself error-free).
    W_eff = (W_hh.astype(np.float64) @ np.eye(H)
             + W_ih.astype(np.float64) @ W_fc.astype(np.float64)).astype(np.float32)
    b0 = (b_ih.astype(np.float64) + b_hh.astype(np.float64)).astype(np.float32)
    b1 = (b_ih.astype(np.float64) + b_hh.astype(np.float64)
          + W_ih.astype(np.float64) @ b_fc.astype(np.float64)).astype(np.float32)

    # SBUF layouts: partition dim = H chunk of 128.
    wg = np.ascontiguousarray(
        W_eff.T.reshape(HC, P, G4).transpose(1, 0, 2)).astype(MM_NP)
    wx = np.zeros((P, G4), MM_NP)
    wx[:O] = W_ih.T.astype(MM_NP)                               # K-padded to 128
    wf = np.ascontiguousarray(
        W_fc.T.reshape(HC, P, O).transpose(1, 0, 2)).astype(MM_NP)
    b0s = np.ascontiguousarray(b0.reshape(G4 // P, P).T)        # [128, 16]
    b1s = np.ascontiguousarray(b1.reshape(G4 // P, P).T)
    bfc = np.ascontiguousarray(b_fc.reshape(O, 1))

    in_maps = []
    for core in range(N_CORES):
        sl = slice(core * NB, (core + 1) * NB)
        h0 = np.ascontiguousarray(
            ctx[sl].T.reshape(HC, P, NB).transpose(1, 0, 2)).astype(MM_NP)
        spc = np.zeros((P, NB), MM_NP)
        # The device uses W_eff for step 0 too, which adds a spurious
        # W_ih@W_fc@context term; feeding sp' = start_pos - context@W_fc.T
        # cancels it exactly (step 0 is linear in its x input).
        sp_eff = (sp[sl].astype(np.float64)
                  - ctx[sl].astype(np.float64) @ W_fc.astype(np.float64).T)
        spc[:O] = sp_eff.T.astype(MM_NP)                        # K-padded to 128
        in_maps.append({
            "h0": h0, "sp": spc, "wg": wg, "wx": wx, "wf": wf,
            "b0": b0s, "b1": b1s, "bfc": bfc,
        })
    return in_maps


def assemble_output(results):
    out = np.empty((N_FULL, T, O), np.float32)
    for core in range(N_CORES):
        pr = results[core]["preds"]                             # [24, 4096]
        out[core * NB:(core + 1) * NB] = (
            pr.reshape(T, O, NB).transpose(2, 0, 1))
    return out


def kernel(**inputs):
    in_maps = prepare_in_maps(inputs)
    nc = _get_nc()
    res = run_bass_kernel_spmd(
        nc, in_maps, core_ids=list(range(N_CORES)), trace=False,
    )
    if res.exec_time_ns is not None:
        kernel.last_exec_time_ns = res.exec_time_ns
    return assemble_output(res.results)



# revision 4
# speedup vs baseline: 1.3239x; 1.3239x over previous
"""DecoderLSTM Trainium2 kernel.

Problem: N=32768 batch rows, H=512 hidden, O=2 outputs, T=12 steps.
    h,c,x = (context, 0, start_pos)
    per step: gates = x @ W_ih.T + h @ W_hh.T + (b_ih + b_hh)
              i,f,g,o = split(gates); c = sig(f)*c + sig(i)*tanh(g)
              h = sig(o)*tanh(c); pred = h @ W_fc.T + b_fc; x = pred
    output: preds stacked [N, T, O]

Key algebraic fold: x_t = W_fc @ h_t + b_fc for t>=1, so
    gates_t = (W_hh + W_ih W_fc) @ h_{t-1} + (b + W_ih b_fc)   (t >= 1)
    gates_0 = W_hh @ context + W_ih @ start_pos + b
which removes the fed-back x input entirely (rank-2 weight correction,
done once on the host).

Sharding: pure data parallel, 4096 batch rows per core, weights replicated.

On-chip layout is "transposed": H lives on partitions (4 chunks of 128),
batch on the free dim, so the recurrence needs no transposes at all:
    gates.T[4H, B] = W_eff @ h.T  via matmul(lhsT=W_eff.T chunk, rhs=h.T chunk)
Per-gate bias is applied for free via the ACT engine's per-partition bias
on the sigmoid/tanh evacuation of PSUM.

Each core processes its 4096 batch cols as 4 chunks of 1024; two chunks
("chains") are interleaved so one chain's matmuls hide the other chain's
activation/elementwise tail between steps.
"""

import os

import numpy as np

import concourse.mybir as mybir
from concourse import bacc
import concourse.tile as tile
from concourse.bass_utils import run_bass_kernel_spmd

P = 128
H = 512
HC = H // P          # 4 H-chunks
G4 = 4 * H           # 2048 gate rows
O = 2
T = 12
N_FULL = 32768
N_CORES = 8
NB = N_FULL // N_CORES   # 4096 batch rows per core
BC = 1024                # batch cols per chain
NCH = NB // BC           # 4 chains per core
F32 = mybir.dt.float32
# float16 runs the PE at full rate (1 col/cycle) with overlapped FWL weight
# loads, and its 10-bit mantissa keeps the recurrence error ~4e-4 of scale.
# float32r (same speed on paper) measured ~20% slower on HW: its self-loading
# weight path serializes against the matmul stream. float32 is exact but 4x.
_MM_DT_NAME = os.environ.get("KERNEL_MM_DT", "float16")
MM_DT = getattr(mybir.dt, _MM_DT_NAME)

# Moving-operand free dim per matmul: one 512-entry PSUM bank.
MMN = 512
NHALF = BC // MMN

# Buffer depths (SBUF is ample at fp16): h ping-pong depth also controls how
# early the next pair's h0 DMA can start; act tiles decouple ACT from DVE.
H_BUFS = int(os.environ.get("KERNEL_HBUFS", "2"))
A_BUFS = int(os.environ.get("KERNEL_ABUFS", "2"))

SIG = mybir.ActivationFunctionType.Sigmoid
TANH = mybir.ActivationFunctionType.Tanh


def build_bass(reps: int = 1):
    # Bacc (not plain Bass): its compile() runs move_matmul_waits_to_ldweights
    # + generate_event_semaphores, which split semaphore waits to the 1-per-
    # instruction hardware limit. Without it, walrus rejects any 2-dep
    # instruction with 'Too many sync wait commands'.
    # reps>1 replays the whole per-core computation; chain_bench uses the
    # T(reps)-slope to cancel dispatch overhead when no NTFF hook exists.
    nc = bacc.Bacc()

    # Tensors feeding the PE are declared in MM_DT end-to-end (for fp32r
    # the BIR verifier additionally requires matmul operands to be *produced*
    # as fp32r, i.e. rounded at the producer).
    h0_d = nc.dram_tensor("h0", [P, HC, NB], MM_DT, kind="ExternalInput")
    sp_d = nc.dram_tensor("sp", [P, NB], MM_DT, kind="ExternalInput")
    wg_d = nc.dram_tensor("wg", [P, HC, G4], MM_DT, kind="ExternalInput")
    wx_d = nc.dram_tensor("wx", [P, G4], MM_DT, kind="ExternalInput")
    wf_d = nc.dram_tensor("wf", [P, HC, O], MM_DT, kind="ExternalInput")
    b0_d = nc.dram_tensor("b0", [P, G4 // P], F32, kind="ExternalInput")
    b1_d = nc.dram_tensor("b1", [P, G4 // P], F32, kind="ExternalInput")
    bfc_d = nc.dram_tensor("bfc", [O, 1], F32, kind="ExternalInput")
    out_d = nc.dram_tensor("preds", [2 * T, NB], F32, kind="ExternalOutput")

    with tile.TileContext(nc) as tc:
        with (
            tc.tile_pool(name="singles", bufs=1) as singles,
            tc.tile_pool(name="state", bufs=1) as state,
            tc.tile_pool(name="acts", bufs=2) as acts,
            tc.tile_pool(name="gpsum", bufs=3, space="PSUM") as gpsum,
            tc.tile_pool(name="ppsum", bufs=1, space="PSUM") as ppsum,
        ):
            wg_sb = singles.tile([P, HC, G4], MM_DT)
            # split by K-chunk so the first matmuls start after ~1/4 of the
            # weights have landed (the only PE idle in the modeled trace is
            # ~16us at startup waiting for these loads)
            for _kj in range(HC):
                nc.sync.dma_start(wg_sb[:, _kj, :], wg_d[:, _kj, :])
            wx_sb = singles.tile([P, G4], MM_DT)
            nc.sync.dma_start(wx_sb[:], wx_d[:])
            wf_sb = singles.tile([P, HC, O], MM_DT)
            nc.sync.dma_start(wf_sb[:], wf_d[:])
            sp_sb = singles.tile([P, NB], MM_DT)
            nc.sync.dma_start(sp_sb[:], sp_d[:])
            b0_sb = singles.tile([P, G4 // P], F32)
            nc.sync.dma_start(b0_sb[:], b0_d[:])
            b1_sb = singles.tile([P, G4 // P], F32)
            nc.sync.dma_start(b1_sb[:], b1_d[:])
            bfc_sb = singles.tile([O, 1], F32)
            nc.sync.dma_start(bfc_sb[:], bfc_d[:])

            st = {}

            def emit_step(s, t):
                """Gates + elementwise for chain s, step t. Updates st[s]."""
                h_prev, c = st[s]
                h_next = state.tile([P, HC, BC], MM_DT, tag=f"h{s % 2}",
                                    bufs=H_BUFS)
                b_sb = b0_sb if t == 0 else b1_sb
                for j in range(HC):
                    gsb = []
                    for g in range(4):
                        mb = 4 * g + j
                        msl = slice(mb * P, (mb + 1) * P)
                        gp = gpsum.tile([P, BC], F32, tag="g")
                        for half in range(NHALF):
                            cs = slice(half * MMN, (half + 1) * MMN)
                            for kj in range(HC):
                                nc.tensor.matmul(
                                    gp[:, cs],
                                    wg_sb[:, kj, msl],
                                    h_prev[:, kj, cs],
                                    start=(kj == 0),
                                    stop=(kj == HC - 1 and t > 0),
                                )
                            if t == 0:
                                scs = slice(s * BC + half * MMN,
                                            s * BC + (half + 1) * MMN)
                                nc.tensor.matmul(
                                    gp[:, cs],
                                    wx_sb[:, msl],
                                    sp_sb[:, scs],
                                    start=False,
                                    stop=True,
                                )
                        a = acts.tile([P, BC], F32, tag=f"a{g}", bufs=A_BUFS)
                        nc.scalar.activation(
                            a[:], gp[:],
                            TANH if g == 2 else SIG,
                            bias=b_sb[:, mb:mb + 1],
                        )
                        gsb.append(a)
                    ai, af, ag, ao = gsb
                    # c_j = f * c_j + i * g ; h_j = o * tanh(c_j)
                    nc.vector.tensor_mul(ai[:], ai[:], ag[:])          # i*g -> ai
                    nc.vector.tensor_mul(c[:, j, :], af[:], c[:, j, :])
                    nc.vector.tensor_add(c[:, j, :], c[:, j, :], ai[:])
                    nc.scalar.activation(ag[:], c[:, j, :], TANH)      # tanh(c) -> ag
                    nc.vector.tensor_mul(h_next[:, j, :], ao[:], ag[:])
                st[s] = (h_next, c)

            def emit_pred(s, t):
                """pred_t = W_fc @ h.T + b_fc -> DRAM rows [2t, 2t+2)."""
                h_cur = st[s][0]
                pp = ppsum.tile([O, NHALF, MMN], F32, tag="pred")
                for half in range(NHALF):
                    cs = slice(half * MMN, (half + 1) * MMN)
                    for kj in range(HC):
                        nc.tensor.matmul(
                            pp[:, half, :],
                            wf_sb[:, kj, :],
                            h_cur[:, kj, cs],
                            start=(kj == 0),
                            stop=(kj == HC - 1),
                        )
                psb = acts.tile([O, NHALF, MMN], F32, tag="pred_sb", bufs=1)
                nc.vector.tensor_scalar_add(psb[:], pp[:], bfc_sb[:, 0:1])
                dst = out_d[2 * t:2 * t + 2, s * BC:(s + 1) * BC]
                nc.sync.dma_start(
                    dst.rearrange("p (h x) -> p h x", h=NHALF), psb[:])

            for pair in [p for _ in range(reps) for p in range(NCH // 2)]:
                chains = [2 * pair, 2 * pair + 1]
                for s in chains:
                    h = state.tile([P, HC, BC], MM_DT, tag=f"h{s % 2}",
                                   bufs=H_BUFS)
                    for _kj in range(HC):
                        nc.sync.dma_start(
                            h[:, _kj, :],
                            h0_d[:, _kj, s * BC:(s + 1) * BC])
                    c = state.tile([P, HC, BC], F32, tag=f"c{s % 2}", bufs=1)
                    nc.vector.memset(c[:], 0.0)
                    st[s] = (h, c)
                a, b = chains
                for t in range(T):
                    emit_step(a, t)
                    if t > 0:
                        emit_pred(b, t - 1)
                    emit_step(b, t)
                    emit_pred(a, t)
                emit_pred(b, T - 1)

    nc.compile()
    return nc


_NC_CACHE = {}


def _get_nc(reps: int = 1):
    key = (_MM_DT_NAME, reps)
    if key not in _NC_CACHE:
        _NC_CACHE[key] = build_bass(reps)
    return _NC_CACHE[key]


MM_NP = mybir.dt.np(MM_DT)


def prepare_in_maps(inputs):
    ctx = np.ascontiguousarray(np.asarray(inputs["context"], dtype=np.float32))
    sp = np.ascontiguousarray(np.asarray(inputs["start_pos"], dtype=np.float32))
    W_ih = np.asarray(inputs["W_ih"], dtype=np.float32)
    W_hh = np.asarray(inputs["W_hh"], dtype=np.float32)
    b_ih = np.asarray(inputs["b_ih"], dtype=np.float32)
    b_hh = np.asarray(inputs["b_hh"], dtype=np.float32)
    W_fc = np.asarray(inputs["W_fc"], dtype=np.float32)
    b_fc = np.asarray(inputs["b_fc"], dtype=np.float32)

    # Fold the fed-back fc layer into the recurrence (exact algebra; done in
    # fp64 to keep the fold itself error-free).
    W_eff = (W_hh.astype(np.float64) @ np.eye(H)
             + W_ih.astype(np.float64) @ W_fc.astype(np.float64)).astype(np.float32)
    b0 = (b_ih.astype(np.float64) + b_hh.astype(np.float64)).astype(np.float32)
    b1 = (b_ih.astype(np.float64) + b_hh.astype(np.float64)
          + W_ih.astype(np.float64) @ b_fc.astype(np.float64)).astype(np.float32)

    # SBUF layouts: partition dim = H chunk of 128.
    wg = np.ascontiguousarray(
        W_eff.T.reshape(HC, P, G4).transpose(1, 0, 2)).astype(MM_NP)
    wx = np.zeros((P, G4), MM_NP)
    wx[:O] = W_ih.T.astype(MM_NP)                               # K-padded to 128
    wf = np.ascontiguousarray(
        W_fc.T.reshape(HC, P, O).transpose(1, 0, 2)).astype(MM_NP)
    b0s = np.ascontiguousarray(b0.reshape(G4 // P, P).T)        # [128, 16]
    b1s = np.ascontiguousarray(b1.reshape(G4 // P, P).T)
    bfc = np.ascontiguousarray(b_fc.reshape(O, 1))

    in_maps = []
    for core in range(N_CORES):
        sl = slice(core * NB, (core + 1) * NB)
        h0 = np.ascontiguousarray(
            ctx[sl].T.reshape(HC, P, NB).transpose(1, 0, 2)).astype(MM_NP)
        spc = np.zeros((P, NB), MM_NP)
        # The device uses W_eff for step 0 too, which adds a spurious
        # W_ih@W_fc@context term; feeding sp' = start_pos - context@W_fc.T
        # cancels it exactly (step 0 is linear in its x input).
        sp_eff = (sp[sl].astype(np.float64)
                  - ctx[sl].astype(np.float64) @ W_fc.astype(np.float64).T)
        spc[:O] = sp_eff.T.astype(MM_NP)                        # K-padded to 128
        in_maps.append({
            "h0": h0, "sp": spc, "wg": wg, "wx": wx, "wf": wf,
            "b0": b0s, "b1": b1s, "bfc": bfc,
        })
    return in_maps


def assemble_output(results):
    out = np.empty((N_FULL, T, O), np.float32)
    for core in range(N_CORES):
        pr = results[core]["preds"]                             # [24, 4096]
        out[core * NB:(core + 1) * NB] = (
            pr.reshape(T, O, NB).transpose(2, 0, 1))
    return out


def kernel(**inputs):
    in_maps = prepare_in_maps(inputs)
    nc = _get_nc()
    res = run_bass_kernel_spmd(
        nc, in_maps, core_ids=list(range(N_CORES)), trace=False,
    )
    if res.exec_time_ns is not None:
        kernel.last_exec_time_ns = res.exec_time_ns
    return assemble_output(res.results)



# revision 6
# speedup vs baseline: 2.8500x; 2.1527x over previous
"""DecoderLSTM Trainium2 kernel (v2: mixed fp8/fp16).

Problem: N=32768 batch rows, H=512 hidden, O=2 outputs, T=12 steps.
    h,c,x = (context, 0, start_pos)
    per step: gates = x @ W_ih.T + h @ W_hh.T + (b_ih + b_hh)
              i,f,g,o = split(gates); c = sig(f)*c + sig(i)*tanh(g)
              h = sig(o)*tanh(c); pred = h @ W_fc.T + b_fc; x = pred
    output: preds stacked [N, T, O]

Algebraic fold (exact, done on host in fp64): x_t = W_fc h_t + b_fc for
t>=1, so the recurrence uses W_eff = W_hh + W_ih W_fc and the fed-back x
disappears; step 0 feeds sp' = start_pos - context @ W_fc.T through W_ih
to cancel the spurious step-0 W_ih W_fc context term.

Precision scheme (validated against the reference in CPU simulation):
  - i/f/o gate matmuls: fp8-e4m3 weights+h, DoubleRow perf mode (2 k-chunks
    per matmul, ~2x PE throughput). Weights pre-scaled by a power of two SW
    so |w|<=~200 (TRN e4m3 max is 240); the PSUM is descaled for free via
    activation(scale=1/SW).
  - g (candidate) gate matmul: fp16 — the tanh path dominates the error
    budget; all-fp8 fails the 2e-2 gate, g-in-fp16 passes with 1.5x margin.
  - elementwise tail (c, gate activations, h) in fp16: 2x DVE mode.
    c never materializes in fp32; step 0 writes c = i*g directly (c0 = 0),
    so there is no memset and no f*c work at t=0.
  - pred path fp16 (W_fc fp16 @ h fp16, fp32 PSUM + bias on DVE).
  h is maintained in BOTH fp16 (g gate + preds) and fp8 (i/f/o gates).

Sharding: pure data parallel, 4096 batch rows per core, weights replicated.

On-chip layout is "transposed": H on partitions (4 chunks of 128), batch on
the free dim, so the recurrence needs no transposes:
    gates.T[4H, B] = W_eff @ h.T  via matmul(lhsT=W_eff.T chunk, rhs=h.T)
Per-gate bias (and the fp8 descale) ride the ACT engine's PSUM evacuation.

Each core processes its 4096 batch cols as 4 chunks of 1024; two chunks
("chains") are interleaved so one chain's matmuls hide the other chain's
activation/elementwise tail between steps.
"""

import numpy as np

import concourse.mybir as mybir
from concourse import bacc
import concourse.tile as tile
from concourse.bass_utils import run_bass_kernel_spmd

P = 128
H = 512
HC = H // P          # 4 k-chunks
O = 2
T = 12
N_FULL = 32768
N_CORES = 8
NB = N_FULL // N_CORES   # 4096 batch rows per core
BC = 1024                # batch cols per chain
NCH = NB // BC           # 4 chains per core
F32 = mybir.dt.float32
F16 = mybir.dt.float16
F8 = mybir.dt.float8e4
DR = mybir.MatmulPerfMode.DoubleRow

# Moving-operand free dim per matmul: one 512-entry PSUM bank.
MMN = 512
NHALF = BC // MMN

# fp8 weight pre-scale (power of two; absmax(W_eff) ~0.05 -> ~200)
SW_TARGET = 200.0

# Gate schedule: (name, original gate row-block, fp8?) — f first so the
# DVE f*c multiply starts as early as possible, o last so tanh(c) lands
# right behind its evacuation on the ACT queue.
GATES = [("f", 1, True), ("i", 0, True), ("g", 2, False), ("o", 3, True)]
IDX8 = {1: 0, 0: 1, 3: 2}      # original gate -> wg8 block index
G8 = len(IDX8) * P * HC        # 1536 fp8 gate rows
G16 = P * HC                   # 512 fp16 (g) gate rows

SIG = mybir.ActivationFunctionType.Sigmoid
TANH = mybir.ActivationFunctionType.Tanh


def build_bass(reps: int = 1):
    # Bacc (not plain Bass): its compile() splits semaphore waits to the
    # 1-per-instruction hardware limit. reps>1 replays the whole per-core
    # computation; chain_bench uses the T(reps)-slope to cancel dispatch
    # overhead when no NTFF hook exists.
    nc = bacc.Bacc()

    h016_d = nc.dram_tensor("h016", [P, HC, NB], F16, kind="ExternalInput")
    h08_d = nc.dram_tensor("h08", [P, HC, NB], F8, kind="ExternalInput")
    sp16_d = nc.dram_tensor("sp16", [P, NB], F16, kind="ExternalInput")
    sp8_d = nc.dram_tensor("sp8", [P, NB], F8, kind="ExternalInput")
    wg16_d = nc.dram_tensor("wg16", [P, HC, G16], F16, kind="ExternalInput")
    wg8_d = nc.dram_tensor("wg8", [P, HC, G8], F8, kind="ExternalInput")
    wx16_d = nc.dram_tensor("wx16", [P, G16], F16, kind="ExternalInput")
    wx8_d = nc.dram_tensor("wx8", [P, G8], F8, kind="ExternalInput")
    wf_d = nc.dram_tensor("wf", [P, HC, O], F16, kind="ExternalInput")
    b0_d = nc.dram_tensor("b0", [P, 16], F32, kind="ExternalInput")
    b1_d = nc.dram_tensor("b1", [P, 16], F32, kind="ExternalInput")
    bfc_d = nc.dram_tensor("bfc", [O, 1], F32, kind="ExternalInput")
    isw_d = nc.dram_tensor("isw", [P, 1], F32, kind="ExternalInput")
    out_d = nc.dram_tensor("preds", [2 * T, NB], F32, kind="ExternalOutput")

    with tile.TileContext(nc) as tc:
        with (
            tc.tile_pool(name="singles", bufs=1) as singles,
            tc.tile_pool(name="state", bufs=1) as state,
            tc.tile_pool(name="acts", bufs=2) as acts,
            tc.tile_pool(name="gpsum", bufs=3, space="PSUM") as gpsum,
            tc.tile_pool(name="ppsum", bufs=1, space="PSUM") as ppsum,
        ):
            wg8_sb = singles.tile([P, HC, G8], F8)
            for _kj in range(HC):
                nc.sync.dma_start(wg8_sb[:, _kj, :], wg8_d[:, _kj, :])
            wg16_sb = singles.tile([P, HC, G16], F16)
            nc.sync.dma_start(wg16_sb[:], wg16_d[:])
            wx8_sb = singles.tile([P, G8], F8)
            nc.sync.dma_start(wx8_sb[:], wx8_d[:])
            wx16_sb = singles.tile([P, G16], F16)
            nc.sync.dma_start(wx16_sb[:], wx16_d[:])
            wf_sb = singles.tile([P, HC, O], F16)
            nc.sync.dma_start(wf_sb[:], wf_d[:])
            b0_sb = singles.tile([P, 16], F32)
            nc.sync.dma_start(b0_sb[:], b0_d[:])
            b1_sb = singles.tile([P, 16], F32)
            nc.sync.dma_start(b1_sb[:], b1_d[:])
            bfc_sb = singles.tile([O, 1], F32)
            nc.sync.dma_start(bfc_sb[:], bfc_d[:])
            isw_sb = singles.tile([P, 1], F32)
            nc.sync.dma_start(isw_sb[:], isw_d[:])
            sp8_sb = singles.tile([P, NB], F8)
            nc.sync.dma_start(sp8_sb[:], sp8_d[:])
            sp16_sb = singles.tile([P, NB], F16)
            nc.sync.dma_start(sp16_sb[:], sp16_d[:])
            # whole h0 stays resident; per-chain DMA split so chain 0's
            # slice lands first and step 0 can start early
            h08_sb = singles.tile([P, HC, NB], F8)
            h016_sb = singles.tile([P, HC, NB], F16)
            for s in range(NCH):
                bsl = slice(s * BC, (s + 1) * BC)
                nc.sync.dma_start(h08_sb[:, :, bsl], h08_d[:, :, bsl])
                nc.sync.dma_start(h016_sb[:, :, bsl], h016_d[:, :, bsl])

            st = {}

            def emit_step(s, t):
                """One recurrence step for chain s. Updates st[s]."""
                h16p, h8p, c = st[s]
                h16n = state.tile([P, HC, BC], F16, tag=f"h16_{s % 2}", bufs=2)
                h8n = state.tile([P, HC, BC], F8, tag=f"h8_{s % 2}", bufs=2)
                if t == 0:
                    c = state.tile([P, HC, BC], F16, tag=f"c{s % 2}", bufs=1)
                b_sb = b0_sb if t == 0 else b1_sb
                for j in range(HC):
                    A = {}
                    for name, og, is8 in GATES:
                        gp = gpsum.tile([P, BC], F32, tag="g")
                        mb = 4 * og + j
                        if is8:
                            m8 = 4 * IDX8[og] + j
                            msl = slice(m8 * P, (m8 + 1) * P)
                            for half in range(NHALF):
                                cs = slice(half * MMN, (half + 1) * MMN)
                                for q in range(HC // 2):
                                    qsl = slice(2 * q, 2 * q + 2)
                                    if t == 0:
                                        rhs = h08_sb[:, qsl,
                                                     s * BC + half * MMN:
                                                     s * BC + (half + 1) * MMN]
                                    else:
                                        rhs = h8p[:, qsl, cs]
                                    nc.tensor.matmul(
                                        gp[:, cs],
                                        wg8_sb[:, qsl, msl],
                                        rhs,
                                        start=(q == 0),
                                        stop=(q == HC // 2 - 1 and t > 0),
                                        perf_mode=DR,
                                    )
                                if t == 0:
                                    scs = slice(s * BC + half * MMN,
                                                s * BC + (half + 1) * MMN)
                                    nc.tensor.matmul(
                                        gp[:, cs],
                                        wx8_sb[:, msl],
                                        sp8_sb[:, scs],
                                        start=False,
                                        stop=True,
                                    )
                        else:
                            msl = slice(j * P, (j + 1) * P)
                            for half in range(NHALF):
                                cs = slice(half * MMN, (half + 1) * MMN)
                                for kj in range(HC):
                                    if t == 0:
                                        rhs = h016_sb[:, kj,
                                                      s * BC + half * MMN:
                                                      s * BC + (half + 1) * MMN]
                                    else:
                                        rhs = h16p[:, kj, cs]
                                    nc.tensor.matmul(
                                        gp[:, cs],
                                        wg16_sb[:, kj, msl],
                                        rhs,
                                        start=(kj == 0),
                                        stop=(kj == HC - 1 and t > 0),
                                    )
                                if t == 0:
                                    scs = slice(s * BC + half * MMN,
                                                s * BC + (half + 1) * MMN)
                                    nc.tensor.matmul(
                                        gp[:, cs],
                                        wx16_sb[:, msl],
                                        sp16_sb[:, scs],
                                        start=False,
                                        stop=True,
                                    )
                        a = acts.tile([P, BC], F16, tag=f"a{name}", bufs=2)
                        nc.scalar.activation(
                            a[:], gp[:],
                            TANH if og == 2 else SIG,
                            bias=b_sb[:, mb:mb + 1],
                            scale=isw_sb[:, 0:1] if is8 else 1.0,
                        )
                        A[name] = a
                    if t == 0:
                        nc.vector.tensor_mul(c[:, j, :], A["i"][:], A["g"][:])
                    else:
                        nc.vector.tensor_mul(A["f"][:], A["f"][:], c[:, j, :])
                        nc.vector.tensor_mul(A["i"][:], A["i"][:], A["g"][:])
                        nc.vector.tensor_add(c[:, j, :], A["f"][:], A["i"][:])
                    th = acts.tile([P, BC], F16, tag=f"th{s % 2}", bufs=2)
                    nc.scalar.activation(th[:], c[:, j, :], TANH)
                    nc.vector.tensor_mul(h16n[:, j, :], A["o"][:], th[:])
                    nc.vector.tensor_copy(h8n[:, j, :], h16n[:, j, :])
                st[s] = (h16n, h8n, c)

            def emit_pred(s, t):
                """pred_t = W_fc @ h.T + b_fc -> DRAM rows [2t, 2t+2)."""
                h16c = st[s][0]
                pp = ppsum.tile([O, NHALF, MMN], F32, tag="pred")
                for half in range(NHALF):
                    cs = slice(half * MMN, (half + 1) * MMN)
                    for kj in range(HC):
                        nc.tensor.matmul(
                            pp[:, half, :],
                            wf_sb[:, kj, :],
                            h16c[:, kj, cs],
                            start=(kj == 0),
                            stop=(kj == HC - 1),
                        )
                psb = acts.tile([O, NHALF, MMN], F32, tag="psb", bufs=2)
                nc.vector.tensor_scalar_add(psb[:], pp[:], bfc_sb[:, 0:1])
                dst = out_d[2 * t:2 * t + 2, s * BC:(s + 1) * BC]
                nc.sync.dma_start(
                    dst.rearrange("p (h x) -> p h x", h=NHALF), psb[:])

            for pair in [p for _ in range(reps) for p in range(NCH // 2)]:
                a, b = 2 * pair, 2 * pair + 1
                st[a] = (None, None, None)
                st[b] = (None, None, None)
                for t in range(T):
                    emit_step(a, t)
                    if t > 0:
                        emit_pred(b, t - 1)
                    emit_step(b, t)
                    emit_pred(a, t)
                emit_pred(b, T - 1)

    nc.compile()
    return nc


_NC_CACHE = {}


def _get_nc(reps: int = 1):
    if reps not in _NC_CACHE:
        _NC_CACHE[reps] = build_bass(reps)
    return _NC_CACHE[reps]


F16_NP = mybir.dt.np(F16)
F8_NP = mybir.dt.np(F8)


def _q8(x, scale=1.0):
    return np.clip(np.asarray(x, np.float32) * scale, -240.0, 240.0).astype(F8_NP)


def prepare_in_maps(inputs):
    ctx = np.ascontiguousarray(np.asarray(inputs["context"], dtype=np.float32))
    sp = np.ascontiguousarray(np.asarray(inputs["start_pos"], dtype=np.float32))
    W_ih = np.asarray(inputs["W_ih"], dtype=np.float64)
    W_hh = np.asarray(inputs["W_hh"], dtype=np.float64)
    b_ih = np.asarray(inputs["b_ih"], dtype=np.float64)
    b_hh = np.asarray(inputs["b_hh"], dtype=np.float64)
    W_fc = np.asarray(inputs["W_fc"], dtype=np.float64)
    b_fc = np.asarray(inputs["b_fc"], dtype=np.float64)

    # Fold the fed-back fc layer into the recurrence (exact algebra in fp64).
    W_eff = (W_hh + W_ih @ W_fc).astype(np.float32)
    b0 = (b_ih + b_hh).astype(np.float32)
    b1 = (b_ih + b_hh + W_ih @ b_fc).astype(np.float32)
    sp_eff = (sp.astype(np.float64) - ctx.astype(np.float64) @ W_fc.T
              ).astype(np.float32)
    Wx = W_ih.astype(np.float32)

    # fp8 weight pre-scale (power of two); PSUM descaled via ACT scale=1/SW.
    rows8 = np.concatenate([W_eff[og * H:(og + 1) * H] for og in IDX8])
    sw = 2.0 ** np.floor(np.log2(
        SW_TARGET / max(np.abs(rows8).max(), np.abs(Wx).max())))

    # W_eff.T chunked [P, kj, gate rows]: wg[p, kj, m] = W_eff[m, kj*128+p]
    WT = np.ascontiguousarray(W_eff.T.reshape(HC, P, 4 * H).transpose(1, 0, 2))
    WxT = np.zeros((P, 4 * H), np.float32)
    WxT[:O] = Wx.T
    wg8 = np.empty((P, HC, G8), F8_NP)
    wx8 = np.empty((P, G8), F8_NP)
    for og, k8 in IDX8.items():
        for j in range(HC):
            src = slice(og * H + j * P, og * H + (j + 1) * P)
            dstm = slice((4 * k8 + j) * P, (4 * k8 + j + 1) * P)
            wg8[:, :, dstm] = _q8(WT[:, :, src], sw)
            wx8[:, dstm] = _q8(WxT[:, src], sw)
    gsl = slice(2 * H, 3 * H)
    wg16 = np.ascontiguousarray(WT[:, :, gsl]).astype(F16_NP)
    wx16 = np.ascontiguousarray(WxT[:, gsl]).astype(F16_NP)
    wf = np.ascontiguousarray(
        W_fc.astype(np.float32).T.reshape(HC, P, O).transpose(1, 0, 2)
    ).astype(F16_NP)
    b0s = np.ascontiguousarray(b0.reshape(16, P).T)
    b1s = np.ascontiguousarray(b1.reshape(16, P).T)
    bfc = np.ascontiguousarray(b_fc.astype(np.float32).reshape(O, 1))
    isw = np.full((P, 1), 1.0 / sw, np.float32)

    in_maps = []
    for core in range(N_CORES):
        sl = slice(core * NB, (core + 1) * NB)
        h0t = np.ascontiguousarray(
            ctx[sl].T.reshape(HC, P, NB).transpose(1, 0, 2))
        spt = np.zeros((P, NB), np.float32)
        spt[:O] = sp_eff[sl].T
        in_maps.append({
            "h016": h0t.astype(F16_NP), "h08": _q8(h0t),
            "sp16": spt.astype(F16_NP), "sp8": _q8(spt),
            "wg16": wg16, "wg8": wg8, "wx16": wx16, "wx8": wx8,
            "wf": wf, "b0": b0s, "b1": b1s, "bfc": bfc, "isw": isw,
        })
    return in_maps


def assemble_output(results):
    out = np.empty((N_FULL, T, O), np.float32)
    for core in range(N_CORES):
        pr = results[core]["preds"]                             # [24, 4096]
        out[core * NB:(core + 1) * NB] = (
            pr.reshape(T, O, NB).transpose(2, 0, 1))
    return out


def kernel(**inputs):
    in_maps = prepare_in_maps(inputs)
    nc = _get_nc()
    res = run_bass_kernel_spmd(
        nc, in_maps, core_ids=list(range(N_CORES)), trace=False,
    )
    if res.exec_time_ns is not None:
        kernel.last_exec_time_ns = res.exec_time_ns
    return assemble_output(res.results)


# revision 7
# speedup vs baseline: 2.9012x; 1.0180x over previous
"""DecoderLSTM Trainium2 kernel (v2: mixed fp8/fp16).

Problem: N=32768 batch rows, H=512 hidden, O=2 outputs, T=12 steps.
    h,c,x = (context, 0, start_pos)
    per step: gates = x @ W_ih.T + h @ W_hh.T + (b_ih + b_hh)
              i,f,g,o = split(gates); c = sig(f)*c + sig(i)*tanh(g)
              h = sig(o)*tanh(c); pred = h @ W_fc.T + b_fc; x = pred
    output: preds stacked [N, T, O]

Algebraic fold (exact, done on host in fp64): x_t = W_fc h_t + b_fc for
t>=1, so the recurrence uses W_eff = W_hh + W_ih W_fc and the fed-back x
disappears; step 0 feeds sp' = start_pos - context @ W_fc.T through W_ih
to cancel the spurious step-0 W_ih W_fc context term.

Precision scheme (validated against the reference in CPU simulation):
  - i/f/o gate matmuls: fp8-e4m3 weights+h, DoubleRow perf mode (2 k-chunks
    per matmul, ~2x PE throughput). Weights pre-scaled by a power of two SW
    so |w|<=~200 (TRN e4m3 max is 240); the PSUM is descaled for free via
    activation(scale=1/SW).
  - g (candidate) gate matmul: fp16 — the tanh path dominates the error
    budget; all-fp8 fails the 2e-2 gate, g-in-fp16 passes with 1.5x margin.
  - elementwise tail (c, gate activations, h) in fp16: 2x DVE mode.
    c never materializes in fp32; step 0 writes c = i*g directly (c0 = 0),
    so there is no memset and no f*c work at t=0.
  - pred path fp16 (W_fc fp16 @ h fp16, fp32 PSUM + bias on DVE).
  h is maintained in BOTH fp16 (g gate + preds) and fp8 (i/f/o gates).

Sharding: pure data parallel, 4096 batch rows per core, weights replicated.

On-chip layout is "transposed": H on partitions (4 chunks of 128), batch on
the free dim, so the recurrence needs no transposes:
    gates.T[4H, B] = W_eff @ h.T  via matmul(lhsT=W_eff.T chunk, rhs=h.T)
Per-gate bias (and the fp8 descale) ride the ACT engine's PSUM evacuation.

Each core processes its 4096 batch cols as 4 chunks of 1024; two chunks
("chains") are interleaved so one chain's matmuls hide the other chain's
activation/elementwise tail between steps.
"""

import numpy as np

import concourse.mybir as mybir
from concourse import bacc
import concourse.tile as tile
from concourse.bass_utils import run_bass_kernel_spmd

P = 128
H = 512
HC = H // P          # 4 k-chunks
O = 2
T = 12
N_FULL = 32768
N_CORES = 8
NB = N_FULL // N_CORES   # 4096 batch rows per core
BC = 1024                # batch cols per chain
NCH = NB // BC           # 4 chains per core
F32 = mybir.dt.float32
F16 = mybir.dt.float16
F8 = mybir.dt.float8e4
DR = mybir.MatmulPerfMode.DoubleRow

# Moving-operand free dim per matmul: one 512-entry PSUM bank.
MMN = 512
NHALF = BC // MMN

# fp8 weight pre-scale (power of two; absmax(W_eff) ~0.05 -> ~200)
SW_TARGET = 200.0

# Gate schedule: (name, original gate row-block, fp8?) — f first so the
# DVE f*c multiply starts as early as possible, o last so tanh(c) lands
# right behind its evacuation on the ACT queue.
GATES = [("f", 1, True), ("i", 0, True), ("g", 2, False), ("o", 3, True)]
IDX8 = {1: 0, 0: 1, 3: 2}      # original gate -> wg8 block index
G8 = len(IDX8) * P * HC        # 1536 fp8 gate rows
G16 = P * HC                   # 512 fp16 (g) gate rows

SIG = mybir.ActivationFunctionType.Sigmoid
TANH = mybir.ActivationFunctionType.Tanh


def build_bass(reps: int = 1):
    # Bacc (not plain Bass): its compile() splits semaphore waits to the
    # 1-per-instruction hardware limit. reps>1 replays the whole per-core
    # computation; chain_bench uses the T(reps)-slope to cancel dispatch
    # overhead when no NTFF hook exists.
    nc = bacc.Bacc()

    h016_d = nc.dram_tensor("h016", [P, HC, NB], F16, kind="ExternalInput")
    h08_d = nc.dram_tensor("h08", [P, HC, NB], F8, kind="ExternalInput")
    sp16_d = nc.dram_tensor("sp16", [P, NB], F16, kind="ExternalInput")
    sp8_d = nc.dram_tensor("sp8", [P, NB], F8, kind="ExternalInput")
    wg16_d = nc.dram_tensor("wg16", [P, HC, G16], F16, kind="ExternalInput")
    wg8_d = nc.dram_tensor("wg8", [P, HC, G8], F8, kind="ExternalInput")
    wx16_d = nc.dram_tensor("wx16", [P, G16], F16, kind="ExternalInput")
    wx8_d = nc.dram_tensor("wx8", [P, G8], F8, kind="ExternalInput")
    wf_d = nc.dram_tensor("wf", [P, HC, O], F16, kind="ExternalInput")
    b0_d = nc.dram_tensor("b0", [P, 16], F32, kind="ExternalInput")
    b1_d = nc.dram_tensor("b1", [P, 16], F32, kind="ExternalInput")
    bfc_d = nc.dram_tensor("bfc", [O, 1], F32, kind="ExternalInput")
    isw_d = nc.dram_tensor("isw", [P, 1], F32, kind="ExternalInput")
    out_d = nc.dram_tensor("preds", [2 * T, NB], F32, kind="ExternalOutput")

    with tile.TileContext(nc) as tc:
        with (
            tc.tile_pool(name="singles", bufs=1) as singles,
            tc.tile_pool(name="state", bufs=1) as state,
            tc.tile_pool(name="acts", bufs=2) as acts,
            tc.tile_pool(name="gpsum", bufs=3, space="PSUM") as gpsum,
            tc.tile_pool(name="ppsum", bufs=1, space="PSUM") as ppsum,
        ):
            wg8_sb = singles.tile([P, HC, G8], F8)
            for _kj in range(HC):
                nc.sync.dma_start(wg8_sb[:, _kj, :], wg8_d[:, _kj, :])
            wg16_sb = singles.tile([P, HC, G16], F16)
            nc.sync.dma_start(wg16_sb[:], wg16_d[:])
            wx8_sb = singles.tile([P, G8], F8)
            nc.sync.dma_start(wx8_sb[:], wx8_d[:])
            wx16_sb = singles.tile([P, G16], F16)
            nc.sync.dma_start(wx16_sb[:], wx16_d[:])
            wf_sb = singles.tile([P, HC, O], F16)
            nc.sync.dma_start(wf_sb[:], wf_d[:])
            b0_sb = singles.tile([P, 16], F32)
            nc.sync.dma_start(b0_sb[:], b0_d[:])
            b1_sb = singles.tile([P, 16], F32)
            nc.sync.dma_start(b1_sb[:], b1_d[:])
            bfc_sb = singles.tile([O, 1], F32)
            nc.sync.dma_start(bfc_sb[:], bfc_d[:])
            isw_sb = singles.tile([P, 1], F32)
            nc.sync.dma_start(isw_sb[:], isw_d[:])
            sp8_sb = singles.tile([P, NB], F8)
            nc.sync.dma_start(sp8_sb[:], sp8_d[:])
            sp16_sb = singles.tile([P, NB], F16)
            nc.sync.dma_start(sp16_sb[:], sp16_d[:])
            # whole h0 stays resident; per-chain DMA split so chain 0's
            # slice lands first and step 0 can start early
            h08_sb = singles.tile([P, HC, NB], F8)
            h016_sb = singles.tile([P, HC, NB], F16)
            for s in range(NCH):
                bsl = slice(s * BC, (s + 1) * BC)
                nc.sync.dma_start(h08_sb[:, :, bsl], h08_d[:, :, bsl])
                nc.sync.dma_start(h016_sb[:, :, bsl], h016_d[:, :, bsl])

            st = {}

            def emit_step(s, t):
                """One recurrence step for chain s. Updates st[s]."""
                h16p, h8p, c = st[s]
                h16n = state.tile([P, HC, BC], F16, tag=f"h16_{s % 2}", bufs=2)
                h8n = state.tile([P, HC, BC], F8, tag=f"h8_{s % 2}", bufs=2)
                if t == 0:
                    c = state.tile([P, HC, BC], F16, tag=f"c{s % 2}", bufs=1)
                b_sb = b0_sb if t == 0 else b1_sb
                AO = {}
                for j in range(HC):
                    A = {}
                    for name, og, is8 in GATES:
                        gp = gpsum.tile([P, BC], F32, tag="g")
                        mb = 4 * og + j
                        if is8:
                            m8 = 4 * IDX8[og] + j
                            msl = slice(m8 * P, (m8 + 1) * P)
                            for half in range(NHALF):
                                cs = slice(half * MMN, (half + 1) * MMN)
                                for q in range(HC // 2):
                                    qsl = slice(2 * q, 2 * q + 2)
                                    if t == 0:
                                        rhs = h08_sb[:, qsl,
                                                     s * BC + half * MMN:
                                                     s * BC + (half + 1) * MMN]
                                    else:
                                        rhs = h8p[:, qsl, cs]
                                    nc.tensor.matmul(
                                        gp[:, cs],
                                        wg8_sb[:, qsl, msl],
                                        rhs,
                                        start=(q == 0),
                                        stop=(q == HC // 2 - 1 and t > 0),
                                        perf_mode=DR,
                                    )
                                if t == 0:
                                    scs = slice(s * BC + half * MMN,
                                                s * BC + (half + 1) * MMN)
                                    nc.tensor.matmul(
                                        gp[:, cs],
                                        wx8_sb[:, msl],
                                        sp8_sb[:, scs],
                                        start=False,
                                        stop=True,
                                    )
                        else:
                            msl = slice(j * P, (j + 1) * P)
                            for half in range(NHALF):
                                cs = slice(half * MMN, (half + 1) * MMN)
                                for kj in range(HC):
                                    if t == 0:
                                        rhs = h016_sb[:, kj,
                                                      s * BC + half * MMN:
                                                      s * BC + (half + 1) * MMN]
                                    else:
                                        rhs = h16p[:, kj, cs]
                                    nc.tensor.matmul(
                                        gp[:, cs],
                                        wg16_sb[:, kj, msl],
                                        rhs,
                                        start=(kj == 0),
                                        stop=(kj == HC - 1 and t > 0),
                                    )
                                if t == 0:
                                    scs = slice(s * BC + half * MMN,
                                                s * BC + (half + 1) * MMN)
                                    nc.tensor.matmul(
                                        gp[:, cs],
                                        wx16_sb[:, msl],
                                        sp16_sb[:, scs],
                                        start=False,
                                        stop=True,
                                    )
                        a = acts.tile([P, BC], F16, tag=f"a{name}",
                                      bufs=3 if name == "o" else 2)
                        nc.scalar.activation(
                            a[:], gp[:],
                            TANH if og == 2 else SIG,
                            bias=b_sb[:, mb:mb + 1],
                            scale=isw_sb[:, 0:1] if is8 else 1.0,
                        )
                        A[name] = a
                    if t == 0:
                        nc.vector.tensor_mul(c[:, j, :], A["i"][:], A["g"][:])
                    else:
                        nc.vector.tensor_mul(A["f"][:], A["f"][:], c[:, j, :])
                        nc.vector.tensor_mul(A["i"][:], A["i"][:], A["g"][:])
                        nc.vector.tensor_add(c[:, j, :], A["f"][:], A["i"][:])
                    AO[j] = A["o"]
                    if j % 2 == 1:
                        # tanh over two j-chunks in one ACT op (c is
                        # contiguous), then h = o*tanh(c) per chunk and one
                        # fused fp16->fp8 cast for both
                        u = j - 1
                        th = acts.tile([P, 2, BC], F16, tag=f"th{s % 2}",
                                       bufs=2)
                        nc.scalar.activation(th[:], c[:, u:u + 2, :], TANH)
                        nc.vector.tensor_mul(
                            h16n[:, u, :], AO[u][:], th[:, 0, :])
                        nc.vector.tensor_mul(
                            h16n[:, u + 1, :], AO[u + 1][:], th[:, 1, :])
                        nc.vector.tensor_copy(
                            h8n[:, u:u + 2, :], h16n[:, u:u + 2, :])
                st[s] = (h16n, h8n, c)

            def emit_pred(s, t):
                """pred_t = W_fc @ h.T + b_fc -> DRAM rows [2t, 2t+2)."""
                h16c = st[s][0]
                pp = ppsum.tile([O, NHALF, MMN], F32, tag="pred")
                for half in range(NHALF):
                    cs = slice(half * MMN, (half + 1) * MMN)
                    for kj in range(HC):
                        nc.tensor.matmul(
                            pp[:, half, :],
                            wf_sb[:, kj, :],
                            h16c[:, kj, cs],
                            start=(kj == 0),
                            stop=(kj == HC - 1),
                        )
                psb = acts.tile([O, NHALF, MMN], F32, tag="psb", bufs=2)
                nc.vector.tensor_scalar_add(psb[:], pp[:], bfc_sb[:, 0:1])
                dst = out_d[2 * t:2 * t + 2, s * BC:(s + 1) * BC]
                nc.sync.dma_start(
                    dst.rearrange("p (h x) -> p h x", h=NHALF), psb[:])

            for pair in [p for _ in range(reps) for p in range(NCH // 2)]:
                a, b = 2 * pair, 2 * pair + 1
                st[a] = (None, None, None)
                st[b] = (None, None, None)
                for t in range(T):
                    emit_step(a, t)
                    if t > 0:
                        emit_pred(b, t - 1)
                    emit_step(b, t)
                    emit_pred(a, t)
                emit_pred(b, T - 1)

    nc.compile()
    return nc


_NC_CACHE = {}


def _get_nc(reps: int = 1):
    if reps not in _NC_CACHE:
        _NC_CACHE[reps] = build_bass(reps)
    return _NC_CACHE[reps]


F16_NP = mybir.dt.np(F16)
F8_NP = mybir.dt.np(F8)


def _q8(x, scale=1.0):
    return np.clip(np.asarray(x, np.float32) * scale, -240.0, 240.0).astype(F8_NP)


def prepare_in_maps(inputs):
    ctx = np.ascontiguousarray(np.asarray(inputs["context"], dtype=np.float32))
    sp = np.ascontiguousarray(np.asarray(inputs["start_pos"], dtype=np.float32))
    W_ih = np.asarray(inputs["W_ih"], dtype=np.float64)
    W_hh = np.asarray(inputs["W_hh"], dtype=np.float64)
    b_ih = np.asarray(inputs["b_ih"], dtype=np.float64)
    b_hh = np.asarray(inputs["b_hh"], dtype=np.float64)
    W_fc = np.asarray(inputs["W_fc"], dtype=np.float64)
    b_fc = np.asarray(inputs["b_fc"], dtype=np.float64)

    # Fold the fed-back fc layer into the recurrence (exact algebra in fp64).
    W_eff = (W_hh + W_ih @ W_fc).astype(np.float32)
    b0 = (b_ih + b_hh).astype(np.float32)
    b1 = (b_ih + b_hh + W_ih @ b_fc).astype(np.float32)
    sp_eff = (sp.astype(np.float64) - ctx.astype(np.float64) @ W_fc.T
              ).astype(np.float32)
    Wx = W_ih.astype(np.float32)

    # fp8 weight pre-scale (power of two); PSUM descaled via ACT scale=1/SW.
    rows8 = np.concatenate([W_eff[og * H:(og + 1) * H] for og in IDX8])
    sw = 2.0 ** np.floor(np.log2(
        SW_TARGET / max(np.abs(rows8).max(), np.abs(Wx).max())))

    # W_eff.T chunked [P, kj, gate rows]: wg[p, kj, m] = W_eff[m, kj*128+p]
    WT = np.ascontiguousarray(W_eff.T.reshape(HC, P, 4 * H).transpose(1, 0, 2))
    WxT = np.zeros((P, 4 * H), np.float32)
    WxT[:O] = Wx.T
    wg8 = np.empty((P, HC, G8), F8_NP)
    wx8 = np.empty((P, G8), F8_NP)
    for og, k8 in IDX8.items():
        for j in range(HC):
            src = slice(og * H + j * P, og * H + (j + 1) * P)
            dstm = slice((4 * k8 + j) * P, (4 * k8 + j + 1) * P)
            wg8[:, :, dstm] = _q8(WT[:, :, src], sw)
            wx8[:, dstm] = _q8(WxT[:, src], sw)
    gsl = slice(2 * H, 3 * H)
    wg16 = np.ascontiguousarray(WT[:, :, gsl]).astype(F16_NP)
    wx16 = np.ascontiguousarray(WxT[:, gsl]).astype(F16_NP)
    wf = np.ascontiguousarray(
        W_fc.astype(np.float32).T.reshape(HC, P, O).transpose(1, 0, 2)
    ).astype(F16_NP)
    b0s = np.ascontiguousarray(b0.reshape(16, P).T)
    b1s = np.ascontiguousarray(b1.reshape(16, P).T)
    bfc = np.ascontiguousarray(b_fc.astype(np.float32).reshape(O, 1))
    isw = np.full((P, 1), 1.0 / sw, np.float32)

    in_maps = []
    for core in range(N_CORES):
        sl = slice(core * NB, (core + 1) * NB)
        h0t = np.ascontiguousarray(
            ctx[sl].T.reshape(HC, P, NB).transpose(1, 0, 2))
        spt = np.zeros((P, NB), np.float32)
        spt[:O] = sp_eff[sl].T
        in_maps.append({
            "h016": h0t.astype(F16_NP), "h08": _q8(h0t),
            "sp16": spt.astype(F16_NP), "sp8": _q8(spt),
            "wg16": wg16, "wg8": wg8, "wx16": wx16, "wx8": wx8,
            "wf": wf, "b0": b0s, "b1": b1s, "bfc": bfc, "isw": isw,
        })
    return in_maps


def assemble_output(results):
    out = np.empty((N_FULL, T, O), np.float32)
    for core in range(N_CORES):
        pr = results[core]["preds"]                             # [24, 4096]
        out[core * NB:(core + 1) * NB] = (
            pr.reshape(T, O, NB).transpose(2, 0, 1))
    return out


def kernel(**inputs):
    in_maps = prepare_in_maps(inputs)
    nc = _get_nc()
    res = run_bass_kernel_spmd(
        nc, in_maps, core_ids=list(range(N_CORES)), trace=False,
    )
    if res.exec_time_ns is not None:
        kernel.last_exec_time_ns = res.exec_time_ns
    return assemble_output(res.results)
